# revision 19
# baseline (speedup 1.0000x reference)
"""Trainium2 Bass kernel: sparse (sliding-window) attention, tensor-parallel
over heads across 8 NeuronCores.

Reference computation (per problem):
  B=1, S=2048, DIM=2880, 64 q heads / 8 kv heads, head_dim 64,
  RoPE (rotate_half, duplicated angles), causal + sliding_window=128 mask,
  logsumexp-sigmoid attention sinks, output projection.

Sharding: core c gets q heads 8c..8c+7 (one kv-head group), wq/wk/wv sliced
column-wise, wo row-wise; each core computes a partial (S, DIM) output which
the host sums (+ wo bias).

Key algebraic simplification: with no max-subtraction in the softmax
(scores are O(+-20) here, safe in fp32),
  out = (softmax(s) @ V) * sigmoid(lse - sink) = (exp(s) @ V) / (l + exp(sink))
where l = sum(exp(s)).  The l column is produced by the PV matmul itself via a
ones-column appended to V.

v2: all operands bf16 (host-side conversion) — halves DMA, removes the
f32->f32r staging copies (DMA lands directly in matmul-ready tiles), runs
small-N matmuls at 1 cyc/row, enables FWL weight loads.  Scores for the
even/odd head halves run as concurrent 64-row tile_position matmuls.
PSUM accumulation stays fp32 throughout.
"""

import math
from contextlib import ExitStack

import numpy as np

import concourse.bass as bass
import concourse.mybir as mybir
import concourse.tile as tile
from concourse import bacc
from concourse.masks import make_identity

F32 = mybir.dt.float32
BF16 = mybir.dt.bfloat16
AF = mybir.ActivationFunctionType

NEG = -1.0e9

# problem constants
S_FULL = 2048
D_FULL = 2880
N_HEADS = 64
N_KV = 8
HD = 64
WINDOW = 128
BLK = 128  # query/key block (must equal WINDOW)
N_CORES = 8
ROPE_FACTOR = 32.0
SCALE = (0.1 * math.log(ROPE_FACTOR) + 1.0) / math.sqrt(HD)


def build_nc(S=S_FULL, D=D_FULL, HQ=N_HEADS // N_CORES, seq_chunk=512):
    """Build the SPMD per-core Bass program.

    Per-core DRAM parameters (host-prepped layouts):
      xT     (D, S)        bf16  input, D on partitions (22.5 128-tiles)
      wqT    (D, HQ*HD)    bf16  scale folded in
      bq     (128, HQ//2)  f32   per-qdim bias columns per 128-row M tile
      wkvT   (D, 128)      bf16  [wk | wv] stacked
      bkv    (128, 1)      f32
      woT    (HQ*HD, D)    bf16
      cosT   (128, S)      bf16  rope cos, rows duplicated x2
      sinTs  (128, S)      bf16  rope sin, signed (-sin for dim<32), dup x2
      mask01 (128, 512)    bf16  0/1 multiplicative mask, scores-transposed
      esinks (128, HQ)     f32   exp(sink_h) replicated down partitions
      out    (S, D)        bf16  output partial
    """
    assert S % BLK == 0 and HQ % 2 == 0
    NQB = S // BLK
    KT = (D + 127) // 128  # D-dim 128-tiles (last may be short)
    NMT = HQ // 2  # 128-row M tiles for Q (2 heads each)
    ADIM = HQ * HD
    seq_chunk = min(seq_chunk, S)
    NSC = (S + seq_chunk - 1) // seq_chunk
    assert S % seq_chunk == 0
    # output D chunks of <=512
    dchunks = []
    off = 0
    while off < D:
        w = min(512, D - off)
        dchunks.append((off, w))
        off += w

    nc = bacc.Bacc(None, target_bir_lowering=False, debug=False)

    xT_d = nc.declare_dram_parameter("xT", [D, S], BF16, isOutput=False)
    wqT_d = nc.declare_dram_parameter("wqT", [D, ADIM], BF16, isOutput=False)
    bq_d = nc.declare_dram_parameter("bq", [128, NMT], F32, isOutput=False)
    wkvT_d = nc.declare_dram_parameter("wkvT", [D, 128], BF16, isOutput=False)
    bkv_d = nc.declare_dram_parameter("bkv", [128, 1], F32, isOutput=False)
    woT_d = nc.declare_dram_parameter("woT", [ADIM, D], BF16, isOutput=False)
    cosT_d = nc.declare_dram_parameter("cosT", [128, S], BF16, isOutput=False)
    sinTs_d = nc.declare_dram_parameter("sinTs", [128, S], BF16, isOutput=False)
    mask01_d = nc.declare_dram_parameter("mask01", [128, 512], BF16, isOutput=False)
    esinks_d = nc.declare_dram_parameter("esinks", [128, HQ], F32, isOutput=False)
    out_d = nc.declare_dram_parameter("out", [S, D], BF16, isOutput=True)

    nfull = D // 128
    rem = D % 128
    GX = 2  # kt-tiles per grouped x DMA (small groups pace PE startup)
    xgroups = [(g, min(GX, nfull - g)) for g in range(0, nfull, GX)]
    GW = 2
    wgroups = [(g, min(GW, nfull - g)) for g in range(0, nfull, GW)]
    GKV = 11
    kvgroups = [(g, min(GKV, nfull - g)) for g in range(0, nfull, GKV)]

    with tile.TileContext(nc) as tc, ExitStack() as ctx:
        # ---------------- persistent pools ----------------
        const = ctx.enter_context(tc.tile_pool(name="const", bufs=1))
        qkpool = ctx.enter_context(tc.tile_pool(name="qkpool", bufs=1))
        psum_proj = ctx.enter_context(tc.tile_pool(name="psum_proj", bufs=2, space="PSUM"))
        psum_s = ctx.enter_context(tc.tile_pool(name="psum_s", bufs=3, space="PSUM"))
        psum_o = ctx.enter_context(tc.tile_pool(name="psum_o", bufs=2, space="PSUM"))
        psum_t = ctx.enter_context(tc.tile_pool(name="psum_t", bufs=1, space="PSUM"))

        cosT = const.tile([128, S], BF16)
        sinTs = const.tile([128, S], BF16)
        mask01 = const.tile([128, 512], BF16)
        esinks = const.tile([128, HQ], F32)
        bq = const.tile([128, NMT], F32)
        bkv = const.tile([128, 1], F32)
        identF = const.tile([128, 128], F32)
        ident = const.tile([128, 128], BF16)
        ones2 = const.tile([128, 2], BF16)

        # persistent activations
        qts = []
        for t in range(NMT):
            qt = qkpool.tile([128, S], BF16, name=f"qt{t}", tag=f"qt{t}")
            qts.append(qt)
        kvt = qkpool.tile([128, S], BF16, name="kvt", tag="kvt")
        kpadO = qkpool.tile([128, S], BF16, name="kpadO", tag="kpadO")
        vaug = []
        for kb in range(NQB):
            va = qkpool.tile([128, HD + 2], BF16, name=f"vaug{kb}", tag=f"vaug{kb}")
            vaug.append(va)
        # wo resident for the whole kernel; DMAs queue behind the phase-1
        # loads and complete long before the first out-projection needs them
        wo_tiles = []
        for t2i in range(ADIM // 128):
            w = qkpool.tile([128, D], BF16, name=f"wo{t2i}", tag=f"wo{t2i}")
            nc.sync.dma_start(out=w, in_=woT_d[t2i * 128 : (t2i + 1) * 128, :])
            wo_tiles.append(w)

        # ---------------- phase 1: QKV projection ----------------
        with tc.tile_pool(name="ph1", bufs=1) as ph1, \
             tc.tile_pool(name="xpool", bufs=1) as xpool, \
             tc.tile_pool(name="ropetmp", bufs=1) as ropetmp:
            # big flat weight/x tiles: col = kt*width + inner
            wqbig = ph1.tile([128, nfull * ADIM], BF16, name="wqbig", tag="wqbig")
            wqlast = ph1.tile([rem, ADIM], BF16, name="wqlast", tag="wqlast") if rem else None
            wkvbig = ph1.tile([128, nfull * 128], BF16, name="wkvbig", tag="wkvbig")
            wkvlast = ph1.tile([rem, 128], BF16, name="wkvlast", tag="wkvlast") if rem else None

            def load_x_chunk(nt):
                xbig = xpool.tile(
                    [128, nfull * seq_chunk], BF16, name=f"xbig{nt}", tag="xbig", bufs=2
                )
                xlast = (
                    xpool.tile([rem, seq_chunk], BF16, name=f"xlast{nt}", tag="xlast", bufs=2)
                    if rem else None
                )
                c0 = nt * seq_chunk
                for gi, (g0, g) in enumerate(xgroups):
                    nc.sync.dma_start(
                        out=xbig[:, g0 * seq_chunk : (g0 + g) * seq_chunk].rearrange(
                            "p (g s) -> p g s", g=g
                        ),
                        in_=xT_d[g0 * 128 : (g0 + g) * 128, c0 : c0 + seq_chunk].rearrange(
                            "(g p) s -> p g s", p=128
                        ),
                    )
                    if nt == 0:
                        load_wq_group(gi)
                if rem:
                    nc.sync.dma_start(
                        out=xlast, in_=xT_d[nfull * 128 : D, c0 : c0 + seq_chunk]
                    )
                return xbig, xlast

            _wq_loaded = set()

            def load_wq_group(gi):
                # interleave wq loads with chunk-0 x loads, one wq group per
                # x group (wgroups and xgroups have the same count)
                if gi >= len(wgroups) or gi in _wq_loaded:
                    return
                _wq_loaded.add(gi)
                g0, g = wgroups[gi]
                nc.sync.dma_start(
                    out=wqbig[:, g0 * ADIM : (g0 + g) * ADIM].rearrange(
                        "p (g m) -> p g m", g=g
                    ),
                    in_=wqT_d[g0 * 128 : (g0 + g) * 128, :].rearrange(
                        "(g p) m -> p g m", p=128
                    ),
                )

            xchunk0 = load_x_chunk(0)  # interleaves wq group loads
            for gi in range(len(wgroups)):
                load_wq_group(gi)
            if rem:
                nc.sync.dma_start(out=wqlast, in_=wqT_d[nfull * 128 : D, :])
            nc.sync.dma_start(out=bq, in_=bq_d[:, :])
            nc.sync.dma_start(out=bkv, in_=bkv_d[:, :])
            for g0, g in kvgroups:
                nc.sync.dma_start(
                    out=wkvbig[:, g0 * 128 : (g0 + g) * 128].rearrange(
                        "p (g m) -> p g m", g=g
                    ),
                    in_=wkvT_d[g0 * 128 : (g0 + g) * 128, :].rearrange(
                        "(g p) m -> p g m", p=128
                    ),
                )
            if rem:
                nc.sync.dma_start(out=wkvlast, in_=wkvT_d[nfull * 128 : D, :])
            nc.sync.dma_start(out=cosT, in_=cosT_d[:, :])
            nc.sync.dma_start(out=sinTs, in_=sinTs_d[:, :])
            nc.sync.dma_start(out=mask01, in_=mask01_d[:, :])
            nc.sync.dma_start(out=esinks, in_=esinks_d[:, :])
            make_identity(nc, identF)
            nc.gpsimd.tensor_copy(ident, identF)
            nc.vector.memset(ones2, 1.0)

            for nt in range(NSC):
                c0 = nt * seq_chunk
                xbig, xlast = xchunk0 if nt == 0 else load_x_chunk(nt)
                # kt-outer / mt-inner: each x tile is consumed right after its
                # DMA lands, with 5 concurrent PSUM accumulation groups
                pss = []
                for mt in range(NMT + 1):
                    pool = psum_proj if mt < 2 else psum_s
                    pss.append(
                        pool.tile(
                            [128, seq_chunk], F32, name=f"psp_{nt}_{mt}",
                            tag="proj" if mt < 2 else "s",
                        )
                    )
                for kt in range(KT):
                    if kt < nfull:
                        rhs = xbig[:, kt * seq_chunk : (kt + 1) * seq_chunk]
                    else:
                        rhs = xlast
                    for mt in range(NMT + 1):
                        if kt < nfull:
                            if mt < NMT:
                                lhs = wqbig[:, kt * ADIM + mt * 128 : kt * ADIM + (mt + 1) * 128]
                            else:
                                lhs = wkvbig[:, kt * 128 : (kt + 1) * 128]
                        else:
                            lhs = wqlast[:, mt * 128 : (mt + 1) * 128] if mt < NMT else wkvlast
                        nc.tensor.matmul(
                            pss[mt], lhs, rhs, start=(kt == 0), stop=(kt == KT - 1)
                        )
                for mt in range(NMT + 1):
                    if mt < NMT:
                        dst = qts[mt]
                        bias = bq[:, mt : mt + 1]
                    else:
                        dst = kvt
                        bias = bkv[:, 0:1]
                    nc.scalar.activation(
                        dst[:, c0 : c0 + seq_chunk], pss[mt], AF.Identity, bias=bias
                    )

                # rope on this seq chunk (swap halves via gpsimd partition-offset copies)
                for t in range(NMT + 1):
                    if t < NMT:
                        src = qts[t]
                        npart = 128
                    else:
                        src = kvt
                        npart = 64  # k rows only
                    sw = ropetmp.tile([128, seq_chunk], BF16, name=f"sw_{nt}_{t}", tag="sw")
                    for base in range(0, npart, 64):
                        nc.gpsimd.tensor_copy(
                            sw[base : base + 32, :],
                            src[base + 32 : base + 64, c0 : c0 + seq_chunk],
                        )
                        nc.gpsimd.tensor_copy(
                            sw[base + 32 : base + 64, :],
                            src[base : base + 32, c0 : c0 + seq_chunk],
                        )
                    t2 = ropetmp.tile([128, seq_chunk], BF16, name=f"t2_{nt}_{t}", tag="t2")
                    nc.vector.tensor_mul(
                        t2[:npart], sw[:npart], sinTs[:npart, c0 : c0 + seq_chunk]
                    )
                    nc.vector.tensor_mul(
                        src[:npart, c0 : c0 + seq_chunk],
                        src[:npart, c0 : c0 + seq_chunk],
                        cosT[:npart, c0 : c0 + seq_chunk],
                    )
                    nc.vector.tensor_add(
                        src[:npart, c0 : c0 + seq_chunk],
                        src[:npart, c0 : c0 + seq_chunk],
                        t2[:npart],
                    )

                # kpadO: K replicated to partitions 64..127, so the odd heads'
                # scores matmul runs as a concurrent (64,0)-row-tile matmul
                cs = slice(c0, c0 + seq_chunk)
                nc.gpsimd.tensor_copy(kpadO[64:128, cs], kvt[0:64, cs])
                # V natural (+ ones cols) per key block in this chunk
                for kb in range(c0 // BLK, (c0 + seq_chunk) // BLK):
                    ptv = psum_t.tile([128, 128], BF16, name=f"vtr{kb}", tag="tr")
                    nc.tensor.transpose(
                        ptv[:, 0:64],
                        kvt[64:128, kb * BLK : (kb + 1) * BLK],
                        ident[64:128, 64:128],
                    )
                    nc.scalar.copy(vaug[kb][:, 0:HD], ptv[:, 0:64])
                    nc.gpsimd.tensor_copy(vaug[kb][:, HD : HD + 2], ones2)

        # ---------------- phase 1.6 + 2 + 3 pools ----------------
        with tc.tile_pool(name="att", bufs=1) as att, \
             tc.tile_pool(name="ppool", bufs=3) as ppool, \
             tc.tile_pool(name="onat_pool", bufs=3) as onat_pool, \
             tc.tile_pool(name="small", bufs=16) as small, \
             tc.tile_pool(name="stage", bufs=2) as stage:

            # ---------------- phase 2+3: attention + out projection ----------------
            p_prev = [None] * (HQ // 2)
            for qb in range(NQB):
                ncols = 256 if qb < NQB - 1 else 128
                onats = []
                for hp in range(HQ // 2):
                    h0 = 2 * hp
                    qtile = qts[hp]
                    # paired scores^T: even head in cols 0:256, odd in 256:512
                    # of pt. The two matmuls run concurrently as 64-row
                    # tile_position groups — their outputs MUST live in
                    # different PSUM banks (same-bank concurrent writes hang
                    # the device).
                    ps_sA = psum_s.tile([128, 256], F32, name=f"sA_{qb}_{hp}", tag="s")
                    ps_sB = psum_s.tile([128, 256], F32, name=f"sB_{qb}_{hp}", tag="s")
                    nc.tensor.matmul(
                        ps_sA[:, 0:ncols],
                        kvt[0:64, qb * BLK : (qb + 1) * BLK],
                        qtile[0:64, qb * BLK : qb * BLK + ncols],
                        start=True,
                        stop=True,
                    )
                    nc.tensor.matmul(
                        ps_sB[:, 0:ncols],
                        kpadO[64:128, qb * BLK : (qb + 1) * BLK],
                        qtile[64:128, qb * BLK : qb * BLK + ncols],
                        start=True,
                        stop=True,
                    )
                    pt = ppool.tile([128, 512], BF16, name=f"p_{qb}_{hp}", tag=f"pp{hp}")
                    for po_, pss_ in ((0, ps_sA), (256, ps_sB)):
                        nc.scalar.activation(
                            pt[:, po_ : po_ + ncols], pss_[:, 0:ncols], AF.Exp
                        )
                        nc.gpsimd.tensor_mul(
                            pt[:, po_ : po_ + ncols],
                            pt[:, po_ : po_ + ncols],
                            mask01[:, po_ : po_ + ncols],
                        )

                    onat = onat_pool.tile(
                        [128, 128], BF16, name=f"on_{qb}_{hp}", tag="onat", bufs=HQ
                    )
                    onats.append(onat)
                    ps_po = psum_o.tile(
                        [128, 2 * (HD + 2)], F32, name=f"o_{qb}_{hp}", tag="o"
                    )
                    for hh in range(2):
                        po = 256 * hh
                        oo = (HD + 2) * hh
                        dst = ps_po[:, oo : oo + HD + 2]
                        if qb > 0:
                            nc.tensor.matmul(
                                dst,
                                p_prev[hp][:, po + 128 : po + 256],
                                vaug[qb - 1],
                                start=True,
                                stop=False,
                            )
                            nc.tensor.matmul(
                                dst,
                                pt[:, po : po + 128],
                                vaug[qb],
                                start=False,
                                stop=True,
                            )
                        else:
                            nc.tensor.matmul(
                                dst,
                                pt[:, po : po + 128],
                                vaug[0],
                                start=True,
                                stop=True,
                            )
                    # denom = l + exp(sink); r = 1/denom; o = o_un * r
                    dn = small.tile([128, 2], F32, name=f"dn_{qb}_{hp}", tag="dn")
                    rr = small.tile([128, 2], F32, name=f"rr_{qb}_{hp}", tag="rr")
                    for hh in range(2):
                        oo = (HD + 2) * hh
                        nc.vector.tensor_add(
                            dn[:, hh : hh + 1],
                            ps_po[:, oo + HD : oo + HD + 1],
                            esinks[:, h0 + hh : h0 + hh + 1],
                        )
                    nc.vector.reciprocal(rr, dn)
                    for hh in range(2):
                        oo = (HD + 2) * hh
                        nc.vector.tensor_scalar_mul(
                            onat[:, 64 * hh : 64 * hh + 64],
                            ps_po[:, oo : oo + HD],
                            rr[:, hh : hh + 1],
                        )
                    p_prev[hp] = pt

                # transpose head pairs into OT layout, then out projection
                ot_cols = []
                for t2i in range(HQ // 2):
                    ptr = psum_t.tile([128, 128], BF16, name=f"otr_{qb}_{t2i}", tag="tr")
                    nc.tensor.transpose(ptr, onats[t2i], ident)
                    otc = onat_pool.tile(
                        [128, 128], BF16, name=f"otc_{qb}_{t2i}", tag="otc", bufs=HQ
                    )
                    nc.scalar.copy(otc, ptr)
                    ot_cols.append(otc)

                ost = stage.tile([128, D], BF16, name=f"ost_{qb}", tag="ost")
                for dc, (doff, dw) in enumerate(dchunks):
                    ps = psum_proj.tile([128, dw], F32, name=f"po_{qb}_{dc}", tag="proj")
                    for t2i in range(HQ // 2):
                        nc.tensor.matmul(
                            ps[:, :dw],
                            ot_cols[t2i],
                            wo_tiles[t2i][:, doff : doff + dw],
                            start=(t2i == 0),
                            stop=(t2i == HQ // 2 - 1),
                        )
                    nc.vector.tensor_copy(ost[:, doff : doff + dw], ps[:, :dw])
                nc.sync.dma_start(out=out_d[qb * BLK : (qb + 1) * BLK, :], in_=ost)

    nc.finalize()
    return nc


def make_core_inputs(x, rope_cache, wq_w, wq_b, wk_w, wk_b, wv_w, wv_b, wo_w,
                     sinks, S=S_FULL, D=D_FULL, HQ=N_HEADS // N_CORES,
                     n_cores=N_CORES):
    """Host-side prep: build the per-core input maps (bf16 activations)."""
    import ml_dtypes

    bf16 = ml_dtypes.bfloat16

    x2 = np.asarray(x, np.float32).reshape(S, D)
    xT = np.ascontiguousarray(x2.T).astype(bf16)

    rc = np.asarray(rope_cache, np.float32)
    cos = rc[:S, :HD].T  # (64, S)
    sin = rc[:S, HD:].T
    cosT = np.ascontiguousarray(np.concatenate([cos, cos], 0)).astype(bf16)
    sgn = np.concatenate([-np.ones((32, 1), np.float32), np.ones((32, 1), np.float32)])
    sinTs = np.ascontiguousarray(np.concatenate([sin * sgn, sin * sgn], 0)).astype(bf16)

    m256 = np.zeros((128, 256), np.float32)
    kk = np.arange(128)[:, None]
    cc = np.arange(128)[None, :]
    m256[:, :128] = np.where(kk <= cc, 1.0, 0.0)
    m256[:, 128:] = np.where(kk > cc, 1.0, 0.0)
    mask01 = np.concatenate([m256, m256], axis=1).astype(bf16)  # (128,512), head pair

    wq_w = np.asarray(wq_w, np.float32)
    wq_b = np.asarray(wq_b, np.float32)
    wk_w = np.asarray(wk_w, np.float32)
    wk_b = np.asarray(wk_b, np.float32)
    wv_w = np.asarray(wv_w, np.float32)
    wv_b = np.asarray(wv_b, np.float32)
    wo_w = np.asarray(wo_w, np.float32)
    sinks = np.asarray(sinks, np.float32)

    ADIM = HQ * HD
    NMT = HQ // 2
    in_maps = []
    for c in range(n_cores):
        qrows = slice(c * ADIM, (c + 1) * ADIM)
        krows = slice(c * HD, (c + 1) * HD)
        wqT = np.ascontiguousarray(wq_w[qrows].T * SCALE).astype(bf16)
        bqv = (wq_b[qrows] * SCALE).reshape(NMT, 128).T  # (128, NMT)
        wkv = np.concatenate([wk_w[krows], wv_w[krows]], 0)  # (128, D)
        wkvT = np.ascontiguousarray(wkv.T).astype(bf16)
        bkv = np.concatenate([wk_b[krows], wv_b[krows]])[:, None]
        woT = np.ascontiguousarray(wo_w[:, qrows].T).astype(bf16)
        es = np.exp(sinks[c * HQ : (c + 1) * HQ])
        esinks = np.repeat(es[None, :], 128, 0)
        in_maps.append(
            {
                "xT": xT,
                "wqT": wqT,
                "bq": np.ascontiguousarray(bqv),
                "wkvT": wkvT,
                "bkv": np.ascontiguousarray(bkv),
                "woT": woT,
                "cosT": cosT,
                "sinTs": sinTs,
                "mask01": mask01,
                "esinks": np.ascontiguousarray(esinks),
            }
        )
    return in_maps


_CACHED = {}


def _make_spmd_runner(nc, in_maps, n_cores):
    """Compile the SPMD program via PJRT (axon) and return
    (run_fn, in_arrays) where run_fn(*arrays) executes on the 8 cores and
    returns per-core output dicts. Outputs are NOT donated (our kernel
    writes every element of out), so the device-resident input arrays can
    be reused across calls for warm-run timing."""
    import jax
    from jax.experimental.shard_map import shard_map
    from jax.sharding import Mesh, NamedSharding, PartitionSpec

    from concourse import bass2jax, mybir as mb

    bass2jax.install_neuronx_cc_hook()
    try:
        import libneuronxla

        if not getattr(libneuronxla, "_err_surfacing", False):
            _inner = libneuronxla.neuronx_cc

            def _wrapped(*a, **kw):
                try:
                    return _inner(*a, **kw)
                except Exception:
                    import traceback

                    traceback.print_exc()
                    raise

            libneuronxla.neuronx_cc = _wrapped
            libneuronxla._err_surfacing = True
    except ImportError:
        pass
    assert nc.dbg_addr is None
    partition_name = nc.partition_id_tensor.name if nc.partition_id_tensor else None

    in_names = []
    out_names = []
    out_avals = []
    zero_outs = []
    for alloc in nc.m.functions[0].allocations:
        if not isinstance(alloc, mb.MemoryLocationSet):
            continue
        name = alloc.memorylocations[0].name
        if alloc.kind == "ExternalInput":
            if name != partition_name:
                in_names.append(name)
        elif alloc.kind == "ExternalOutput":
            out_names.append(name)
            shape = tuple(alloc.tensor_shape)
            dtype = mb.dt.np(alloc.dtype)
            out_avals.append(jax.core.ShapedArray(shape, dtype))
            zero_outs.append(np.zeros(shape, dtype))
    n_params = len(in_names)
    all_names = in_names + out_names
    if partition_name is not None:
        all_names = all_names + [partition_name]

    def _body(*args):
        operands = list(args)
        if partition_name is not None:
            operands.append(bass2jax.partition_id_tensor())
        outs = bass2jax._bass_exec_p.bind(
            *operands,
            out_avals=tuple(out_avals),
            in_names=tuple(all_names),
            out_names=tuple(out_names),
            lowering_input_output_aliases=(),
            sim_require_finite=True,
            sim_require_nnan=True,
            nc=nc,
        )
        return tuple(outs)

    devices = jax.devices()[:n_cores]
    mesh = Mesh(np.asarray(devices), ("core",))
    sharded = jax.jit(
        shard_map(
            _body,
            mesh=mesh,
            in_specs=(PartitionSpec("core"),) * (n_params + len(out_names)),
            out_specs=(PartitionSpec("core"),) * len(out_names),
            check_rep=False,
        ),
        keep_unused=True,
    )
    sh = NamedSharding(mesh, PartitionSpec("core"))
    arrs = []
    for i, name in enumerate(in_names):
        cat = np.concatenate([m[name] for m in in_maps], axis=0)
        arrs.append(jax.device_put(cat, sh))
    for z in zero_outs:
        cat = np.zeros((n_cores * z.shape[0], *z.shape[1:]), z.dtype)
        arrs.append(jax.device_put(cat, sh))

    def run():
        import jax as _jax

        return _jax.block_until_ready(sharded(*arrs))

    run.async_call = lambda: sharded(*arrs)

    def unpack(out_arrs):
        return [
            {
                name: np.asarray(out_arrs[i]).reshape(n_cores, *out_avals[i].shape)[c]
                for i, name in enumerate(out_names)
            }
            for c in range(n_cores)
        ]

    return run, unpack


def _tiny_nc():
    """Minimal 8-core program to measure the dispatch/RTT floor."""
    nc = bacc.Bacc(None, target_bir_lowering=False, debug=False)
    a = nc.declare_dram_parameter("a", [128, 128], F32, isOutput=False)
    b = nc.declare_dram_parameter("b", [128, 128], F32, isOutput=True)
    with tile.TileContext(nc) as tc, ExitStack() as ctx:
        pool = ctx.enter_context(tc.tile_pool(name="p", bufs=1))
        t = pool.tile([128, 128], F32)
        nc.sync.dma_start(out=t, in_=a[:, :])
        nc.sync.dma_start(out=b[:, :], in_=t)
    nc.finalize()
    return nc


def measure_overhead_ns(n_warm=10):
    import time

    nc = _tiny_nc()
    in_maps = [{"a": np.zeros((128, 128), np.float32)} for _ in range(N_CORES)]
    run, _ = _make_spmd_runner(nc, in_maps, N_CORES)
    run()
    best = float("inf")
    for _ in range(n_warm):
        t0 = time.perf_counter()
        run()
        best = min(best, time.perf_counter() - t0)
    return best * 1e9


def kernel(x, rope_cache, wq_w, wq_b, wk_w, wk_b, wv_w, wv_b, wo_w, wo_b,
           sinks, sliding_window, _time_runs=0):
    import time

    in_maps = make_core_inputs(
        x, rope_cache, wq_w, wq_b, wk_w, wk_b, wv_w, wv_b, wo_w, sinks
    )
    if "nc" not in _CACHED:
        _CACHED["nc"] = build_nc()
    nc = _CACHED["nc"]
    run, unpack = _make_spmd_runner(nc, in_maps, N_CORES)
    _CACHED["run"] = run
    out_arrs = run()  # compile + first run
    if _time_runs:
        best = float("inf")
        for _ in range(_time_runs):
            t0 = time.perf_counter()
            out_arrs = run()
            best = min(best, time.perf_counter() - t0)
        kernel.last_wall_ns = best * 1e9
    else:
        kernel.last_wall_ns = None
    res = unpack(out_arrs)
    out = None
    for r in res:
        o = np.asarray(r["out"], dtype=np.float32)
        out = o if out is None else out + o
    out = out + np.asarray(wo_b, np.float32)[None, :]
    return out.reshape(1, S_FULL, D_FULL).astype(np.float32)


kernel.last_wall_ns = None


# revision 20
# speedup vs baseline: 2.3616x; 2.3616x over previous
"""Trainium2 Bass kernel: sparse (sliding-window) attention, tensor-parallel
over heads across 8 NeuronCores.

Reference computation (per problem):
  B=1, S=2048, DIM=2880, 64 q heads / 8 kv heads, head_dim 64,
  RoPE (rotate_half, duplicated angles), causal + sliding_window=128 mask,
  logsumexp-sigmoid attention sinks, output projection.

Sharding: core c gets q heads 8c..8c+7 (one kv-head group), wq/wk/wv sliced
column-wise, wo row-wise; each core computes a partial (S, DIM) output which
the host sums (+ wo bias).

Key algebraic simplification: with no max-subtraction in the softmax
(scores are O(+-20) here, safe in fp32),
  out = (softmax(s) @ V) * sigmoid(lse - sink) = (exp(s) @ V) / (l + exp(sink))
where l = sum(exp(s)).  The l column is produced by the PV matmul itself via a
ones-column appended to V.

v2: all operands bf16 (host-side conversion) — halves DMA, removes the
f32->f32r staging copies (DMA lands directly in matmul-ready tiles), runs
small-N matmuls at 1 cyc/row, enables FWL weight loads.  Scores for the
even/odd head halves run as concurrent 64-row tile_position matmuls.
PSUM accumulation stays fp32 throughout.
"""

import math
from contextlib import ExitStack

import numpy as np

import concourse.bass as bass
import concourse.mybir as mybir
import concourse.tile as tile
from concourse import bacc
from concourse.masks import make_identity

F32 = mybir.dt.float32
BF16 = mybir.dt.bfloat16
AF = mybir.ActivationFunctionType

NEG = -1.0e9

# problem constants
S_FULL = 2048
D_FULL = 2880
N_HEADS = 64
N_KV = 8
HD = 64
WINDOW = 128
BLK = 128  # query/key block (must equal WINDOW)
N_CORES = 8
ROPE_FACTOR = 32.0
SCALE = (0.1 * math.log(ROPE_FACTOR) + 1.0) / math.sqrt(HD)


def build_nc(S=S_FULL, D=D_FULL, HQ=N_HEADS // N_CORES, seq_chunk=512):
    """Build the SPMD per-core Bass program.

    Per-core DRAM parameters (host-prepped layouts):
      xT     (D, S)        bf16  input, D on partitions (22.5 128-tiles)
      wqT    (D, HQ*HD)    bf16  scale folded in
      bq     (128, HQ//2)  f32   per-qdim bias columns per 128-row M tile
      wkvT   (D, 128)      bf16  [wk | wv] stacked
      bkv    (128, 1)      f32
      woT    (HQ*HD, D)    bf16
      cosT   (128, S)      bf16  rope cos, rows duplicated x2
      sinTs  (128, S)      bf16  rope sin, signed (-sin for dim<32), dup x2
      mask01 (128, 512)    bf16  0/1 multiplicative mask, scores-transposed
      esinks (128, HQ)     f32   exp(sink_h) replicated down partitions
      out    (S, D)        bf16  output partial
    """
    assert S % BLK == 0 and HQ % 2 == 0
    NQB = S // BLK
    KT = (D + 127) // 128  # D-dim 128-tiles (last may be short)
    NMT = HQ // 2  # 128-row M tiles for Q (2 heads each)
    ADIM = HQ * HD
    seq_chunk = min(seq_chunk, S)
    NSC = (S + seq_chunk - 1) // seq_chunk
    assert S % seq_chunk == 0
    # output D chunks of <=512
    dchunks = []
    off = 0
    while off < D:
        w = min(512, D - off)
        dchunks.append((off, w))
        off += w

    nc = bacc.Bacc(None, target_bir_lowering=False, debug=False)

    xT_d = nc.declare_dram_parameter("xT", [D, S], BF16, isOutput=False)
    wqT_d = nc.declare_dram_parameter("wqT", [D, ADIM], BF16, isOutput=False)
    wkvT_d = nc.declare_dram_parameter("wkvT", [D, 128], BF16, isOutput=False)
    # cf packs [bq (NMT) | bkv (1) | esinks (HQ)] f32 columns
    cf_d = nc.declare_dram_parameter("cf", [128, NMT + 1 + HQ], F32, isOutput=False)
    woT_d = nc.declare_dram_parameter("woT", [ADIM, D], BF16, isOutput=False)
    # csall packs [cosT | sinTs | mask01] to cut per-call dispatch args
    csall_d = nc.declare_dram_parameter("csall", [128, 2 * S + 512], BF16, isOutput=False)
    out_d = nc.declare_dram_parameter("out", [S, D], BF16, isOutput=True)

    nfull = D // 128
    rem = D % 128
    GX = 2  # kt-tiles per grouped x DMA (small groups pace PE startup)
    xgroups = [(g, min(GX, nfull - g)) for g in range(0, nfull, GX)]
    GW = 2
    wgroups = [(g, min(GW, nfull - g)) for g in range(0, nfull, GW)]
    GKV = 11
    kvgroups = [(g, min(GKV, nfull - g)) for g in range(0, nfull, GKV)]

    with tile.TileContext(nc) as tc, ExitStack() as ctx:
        # ---------------- persistent pools ----------------
        const = ctx.enter_context(tc.tile_pool(name="const", bufs=1))
        qkpool = ctx.enter_context(tc.tile_pool(name="qkpool", bufs=1))
        psum_proj = ctx.enter_context(tc.tile_pool(name="psum_proj", bufs=2, space="PSUM"))
        psum_s = ctx.enter_context(tc.tile_pool(name="psum_s", bufs=3, space="PSUM"))
        psum_o = ctx.enter_context(tc.tile_pool(name="psum_o", bufs=2, space="PSUM"))
        psum_t = ctx.enter_context(tc.tile_pool(name="psum_t", bufs=1, space="PSUM"))

        csall = const.tile([128, 2 * S + 512], BF16)
        cosT = csall[:, 0:S]
        sinTs = csall[:, S : 2 * S]
        mask01 = csall[:, 2 * S : 2 * S + 512]
        cf = const.tile([128, NMT + 1 + HQ], F32)
        bq = cf[:, 0:NMT]
        bkv = cf[:, NMT : NMT + 1]
        esinks = cf[:, NMT + 1 : NMT + 1 + HQ]
        identF = const.tile([128, 128], F32)
        ident = const.tile([128, 128], BF16)
        ones2 = const.tile([128, 2], BF16)

        # persistent activations
        qts = []
        for t in range(NMT):
            qt = qkpool.tile([128, S], BF16, name=f"qt{t}", tag=f"qt{t}")
            qts.append(qt)
        kvt = qkpool.tile([128, S], BF16, name="kvt", tag="kvt")
        kpadO = qkpool.tile([128, S], BF16, name="kpadO", tag="kpadO")
        vaug = []
        for kb in range(NQB):
            va = qkpool.tile([128, HD + 2], BF16, name=f"vaug{kb}", tag=f"vaug{kb}")
            vaug.append(va)
        # wo resident for the whole kernel; DMAs queue behind the phase-1
        # loads and complete long before the first out-projection needs them
        wo_tiles = []
        for t2i in range(ADIM // 128):
            w = qkpool.tile([128, D], BF16, name=f"wo{t2i}", tag=f"wo{t2i}")
            nc.sync.dma_start(out=w, in_=woT_d[t2i * 128 : (t2i + 1) * 128, :])
            wo_tiles.append(w)

        # ---------------- phase 1: QKV projection ----------------
        with tc.tile_pool(name="ph1", bufs=1) as ph1, \
             tc.tile_pool(name="xpool", bufs=1) as xpool, \
             tc.tile_pool(name="ropetmp", bufs=1) as ropetmp:
            # big flat weight/x tiles: col = kt*width + inner
            wqbig = ph1.tile([128, nfull * ADIM], BF16, name="wqbig", tag="wqbig")
            wqlast = ph1.tile([rem, ADIM], BF16, name="wqlast", tag="wqlast") if rem else None
            wkvbig = ph1.tile([128, nfull * 128], BF16, name="wkvbig", tag="wkvbig")
            wkvlast = ph1.tile([rem, 128], BF16, name="wkvlast", tag="wkvlast") if rem else None

            def load_x_chunk(nt):
                xbig = xpool.tile(
                    [128, nfull * seq_chunk], BF16, name=f"xbig{nt}", tag="xbig", bufs=2
                )
                xlast = (
                    xpool.tile([rem, seq_chunk], BF16, name=f"xlast{nt}", tag="xlast", bufs=2)
                    if rem else None
                )
                c0 = nt * seq_chunk
                for gi, (g0, g) in enumerate(xgroups):
                    nc.sync.dma_start(
                        out=xbig[:, g0 * seq_chunk : (g0 + g) * seq_chunk].rearrange(
                            "p (g s) -> p g s", g=g
                        ),
                        in_=xT_d[g0 * 128 : (g0 + g) * 128, c0 : c0 + seq_chunk].rearrange(
                            "(g p) s -> p g s", p=128
                        ),
                    )
                    if nt == 0:
                        load_wq_group(gi)
                if rem:
                    nc.sync.dma_start(
                        out=xlast, in_=xT_d[nfull * 128 : D, c0 : c0 + seq_chunk]
                    )
                return xbig, xlast

            _wq_loaded = set()

            def load_wq_group(gi):
                # interleave wq loads with chunk-0 x loads, one wq group per
                # x group (wgroups and xgroups have the same count)
                if gi >= len(wgroups) or gi in _wq_loaded:
                    return
                _wq_loaded.add(gi)
                g0, g = wgroups[gi]
                nc.sync.dma_start(
                    out=wqbig[:, g0 * ADIM : (g0 + g) * ADIM].rearrange(
                        "p (g m) -> p g m", g=g
                    ),
                    in_=wqT_d[g0 * 128 : (g0 + g) * 128, :].rearrange(
                        "(g p) m -> p g m", p=128
                    ),
                )

            xchunk0 = load_x_chunk(0)  # interleaves wq group loads
            for gi in range(len(wgroups)):
                load_wq_group(gi)
            if rem:
                nc.sync.dma_start(out=wqlast, in_=wqT_d[nfull * 128 : D, :])
            nc.sync.dma_start(out=cf, in_=cf_d[:, :])
            for g0, g in kvgroups:
                nc.sync.dma_start(
                    out=wkvbig[:, g0 * 128 : (g0 + g) * 128].rearrange(
                        "p (g m) -> p g m", g=g
                    ),
                    in_=wkvT_d[g0 * 128 : (g0 + g) * 128, :].rearrange(
                        "(g p) m -> p g m", p=128
                    ),
                )
            if rem:
                nc.sync.dma_start(out=wkvlast, in_=wkvT_d[nfull * 128 : D, :])
            nc.sync.dma_start(out=csall, in_=csall_d[:, :])
            make_identity(nc, identF)
            nc.gpsimd.tensor_copy(ident, identF)
            nc.vector.memset(ones2, 1.0)

            for nt in range(NSC):
                c0 = nt * seq_chunk
                xbig, xlast = xchunk0 if nt == 0 else load_x_chunk(nt)
                # kt-outer / mt-inner: each x tile is consumed right after its
                # DMA lands, with 5 concurrent PSUM accumulation groups
                pss = []
                for mt in range(NMT + 1):
                    pool = psum_proj if mt < 2 else psum_s
                    pss.append(
                        pool.tile(
                            [128, seq_chunk], F32, name=f"psp_{nt}_{mt}",
                            tag="proj" if mt < 2 else "s",
                        )
                    )
                for kt in range(KT):
                    if kt < nfull:
                        rhs = xbig[:, kt * seq_chunk : (kt + 1) * seq_chunk]
                    else:
                        rhs = xlast
                    for mt in range(NMT + 1):
                        if kt < nfull:
                            if mt < NMT:
                                lhs = wqbig[:, kt * ADIM + mt * 128 : kt * ADIM + (mt + 1) * 128]
                            else:
                                lhs = wkvbig[:, kt * 128 : (kt + 1) * 128]
                        else:
                            lhs = wqlast[:, mt * 128 : (mt + 1) * 128] if mt < NMT else wkvlast
                        nc.tensor.matmul(
                            pss[mt], lhs, rhs, start=(kt == 0), stop=(kt == KT - 1)
                        )
                for mt in range(NMT + 1):
                    if mt < NMT:
                        dst = qts[mt]
                        bias = bq[:, mt : mt + 1]
                    else:
                        dst = kvt
                        bias = bkv[:, 0:1]
                    nc.scalar.activation(
                        dst[:, c0 : c0 + seq_chunk], pss[mt], AF.Identity, bias=bias
                    )

                # rope on this seq chunk (swap halves via gpsimd partition-offset copies)
                for t in range(NMT + 1):
                    if t < NMT:
                        src = qts[t]
                        npart = 128
                    else:
                        src = kvt
                        npart = 64  # k rows only
                    sw = ropetmp.tile([128, seq_chunk], BF16, name=f"sw_{nt}_{t}", tag="sw")
                    for base in range(0, npart, 64):
                        nc.gpsimd.tensor_copy(
                            sw[base : base + 32, :],
                            src[base + 32 : base + 64, c0 : c0 + seq_chunk],
                        )
                        nc.gpsimd.tensor_copy(
                            sw[base + 32 : base + 64, :],
                            src[base : base + 32, c0 : c0 + seq_chunk],
                        )
                    t2 = ropetmp.tile([128, seq_chunk], BF16, name=f"t2_{nt}_{t}", tag="t2")
                    nc.vector.tensor_mul(
                        t2[:npart], sw[:npart], sinTs[:npart, c0 : c0 + seq_chunk]
                    )
                    nc.vector.tensor_mul(
                        src[:npart, c0 : c0 + seq_chunk],
                        src[:npart, c0 : c0 + seq_chunk],
                        cosT[:npart, c0 : c0 + seq_chunk],
                    )
                    nc.vector.tensor_add(
                        src[:npart, c0 : c0 + seq_chunk],
                        src[:npart, c0 : c0 + seq_chunk],
                        t2[:npart],
                    )

                # kpadO: K replicated to partitions 64..127, so the odd heads'
                # scores matmul runs as a concurrent (64,0)-row-tile matmul
                cs = slice(c0, c0 + seq_chunk)
                nc.gpsimd.tensor_copy(kpadO[64:128, cs], kvt[0:64, cs])
                # V natural (+ ones cols) per key block in this chunk
                for kb in range(c0 // BLK, (c0 + seq_chunk) // BLK):
                    ptv = psum_t.tile([128, 128], BF16, name=f"vtr{kb}", tag="tr")
                    nc.tensor.transpose(
                        ptv[:, 0:64],
                        kvt[64:128, kb * BLK : (kb + 1) * BLK],
                        ident[64:128, 64:128],
                    )
                    nc.scalar.copy(vaug[kb][:, 0:HD], ptv[:, 0:64])
                    nc.gpsimd.tensor_copy(vaug[kb][:, HD : HD + 2], ones2)

        # ---------------- phase 1.6 + 2 + 3 pools ----------------
        with tc.tile_pool(name="att", bufs=1) as att, \
             tc.tile_pool(name="ppool", bufs=3) as ppool, \
             tc.tile_pool(name="onat_pool", bufs=3) as onat_pool, \
             tc.tile_pool(name="small", bufs=16) as small, \
             tc.tile_pool(name="stage", bufs=2) as stage:

            # ---------------- phase 2+3: attention + out projection ----------------
            p_prev = [None] * (HQ // 2)
            for qb in range(NQB):
                ncols = 256 if qb < NQB - 1 else 128
                onats = []
                for hp in range(HQ // 2):
                    h0 = 2 * hp
                    qtile = qts[hp]
                    # paired scores^T: even head in cols 0:256, odd in 256:512
                    # of pt. The two matmuls run concurrently as 64-row
                    # tile_position groups — their outputs MUST live in
                    # different PSUM banks (same-bank concurrent writes hang
                    # the device).
                    ps_sA = psum_s.tile([128, 256], F32, name=f"sA_{qb}_{hp}", tag="s")
                    ps_sB = psum_s.tile([128, 256], F32, name=f"sB_{qb}_{hp}", tag="s")
                    nc.tensor.matmul(
                        ps_sA[:, 0:ncols],
                        kvt[0:64, qb * BLK : (qb + 1) * BLK],
                        qtile[0:64, qb * BLK : qb * BLK + ncols],
                        start=True,
                        stop=True,
                    )
                    nc.tensor.matmul(
                        ps_sB[:, 0:ncols],
                        kpadO[64:128, qb * BLK : (qb + 1) * BLK],
                        qtile[64:128, qb * BLK : qb * BLK + ncols],
                        start=True,
                        stop=True,
                    )
                    pt = ppool.tile([128, 512], BF16, name=f"p_{qb}_{hp}", tag=f"pp{hp}")
                    for po_, pss_ in ((0, ps_sA), (256, ps_sB)):
                        nc.scalar.activation(
                            pt[:, po_ : po_ + ncols], pss_[:, 0:ncols], AF.Exp
                        )
                        nc.gpsimd.tensor_mul(
                            pt[:, po_ : po_ + ncols],
                            pt[:, po_ : po_ + ncols],
                            mask01[:, po_ : po_ + ncols],
                        )

                    onat = onat_pool.tile(
                        [128, 128], BF16, name=f"on_{qb}_{hp}", tag="onat", bufs=HQ
                    )
                    onats.append(onat)
                    ps_po = psum_o.tile(
                        [128, 2 * (HD + 2)], F32, name=f"o_{qb}_{hp}", tag="o"
                    )
                    for hh in range(2):
                        po = 256 * hh
                        oo = (HD + 2) * hh
                        dst = ps_po[:, oo : oo + HD + 2]
                        if qb > 0:
                            nc.tensor.matmul(
                                dst,
                                p_prev[hp][:, po + 128 : po + 256],
                                vaug[qb - 1],
                                start=True,
                                stop=False,
                            )
                            nc.tensor.matmul(
                                dst,
                                pt[:, po : po + 128],
                                vaug[qb],
                                start=False,
                                stop=True,
                            )
                        else:
                            nc.tensor.matmul(
                                dst,
                                pt[:, po : po + 128],
                                vaug[0],
                                start=True,
                                stop=True,
                            )
                    # denom = l + exp(sink); r = 1/denom; o = o_un * r
                    dn = small.tile([128, 2], F32, name=f"dn_{qb}_{hp}", tag="dn")
                    rr = small.tile([128, 2], F32, name=f"rr_{qb}_{hp}", tag="rr")
                    for hh in range(2):
                        oo = (HD + 2) * hh
                        nc.vector.tensor_add(
                            dn[:, hh : hh + 1],
                            ps_po[:, oo + HD : oo + HD + 1],
                            esinks[:, h0 + hh : h0 + hh + 1],
                        )
                    nc.vector.reciprocal(rr, dn)
                    for hh in range(2):
                        oo = (HD + 2) * hh
                        nc.vector.tensor_scalar_mul(
                            onat[:, 64 * hh : 64 * hh + 64],
                            ps_po[:, oo : oo + HD],
                            rr[:, hh : hh + 1],
                        )
                    p_prev[hp] = pt

                # transpose head pairs into OT layout, then out projection
                ot_cols = []
                for t2i in range(HQ // 2):
                    ptr = psum_t.tile([128, 128], BF16, name=f"otr_{qb}_{t2i}", tag="tr")
                    nc.tensor.transpose(ptr, onats[t2i], ident)
                    otc = onat_pool.tile(
                        [128, 128], BF16, name=f"otc_{qb}_{t2i}", tag="otc", bufs=HQ
                    )
                    nc.scalar.copy(otc, ptr)
                    ot_cols.append(otc)

                ost = stage.tile([128, D], BF16, name=f"ost_{qb}", tag="ost")
                for dc, (doff, dw) in enumerate(dchunks):
                    ps = psum_proj.tile([128, dw], F32, name=f"po_{qb}_{dc}", tag="proj")
                    for t2i in range(HQ // 2):
                        nc.tensor.matmul(
                            ps[:, :dw],
                            ot_cols[t2i],
                            wo_tiles[t2i][:, doff : doff + dw],
                            start=(t2i == 0),
                            stop=(t2i == HQ // 2 - 1),
                        )
                    nc.vector.tensor_copy(ost[:, doff : doff + dw], ps[:, :dw])
                nc.sync.dma_start(out=out_d[qb * BLK : (qb + 1) * BLK, :], in_=ost)

    nc.finalize()
    return nc


def make_core_inputs(x, rope_cache, wq_w, wq_b, wk_w, wk_b, wv_w, wv_b, wo_w,
                     sinks, S=S_FULL, D=D_FULL, HQ=N_HEADS // N_CORES,
                     n_cores=N_CORES):
    """Host-side prep: build the per-core input maps (bf16 activations)."""
    import ml_dtypes

    bf16 = ml_dtypes.bfloat16

    x2 = np.asarray(x, np.float32).reshape(S, D)
    xT = np.ascontiguousarray(x2.T).astype(bf16)

    rc = np.asarray(rope_cache, np.float32)
    cos = rc[:S, :HD].T  # (64, S)
    sin = rc[:S, HD:].T
    cosT = np.ascontiguousarray(np.concatenate([cos, cos], 0)).astype(bf16)
    sgn = np.concatenate([-np.ones((32, 1), np.float32), np.ones((32, 1), np.float32)])
    sinTs = np.ascontiguousarray(np.concatenate([sin * sgn, sin * sgn], 0)).astype(bf16)

    m256 = np.zeros((128, 256), np.float32)
    kk = np.arange(128)[:, None]
    cc = np.arange(128)[None, :]
    m256[:, :128] = np.where(kk <= cc, 1.0, 0.0)
    m256[:, 128:] = np.where(kk > cc, 1.0, 0.0)
    mask01 = np.concatenate([m256, m256], axis=1).astype(bf16)  # (128,512), head pair

    wq_w = np.asarray(wq_w, np.float32)
    wq_b = np.asarray(wq_b, np.float32)
    wk_w = np.asarray(wk_w, np.float32)
    wk_b = np.asarray(wk_b, np.float32)
    wv_w = np.asarray(wv_w, np.float32)
    wv_b = np.asarray(wv_b, np.float32)
    wo_w = np.asarray(wo_w, np.float32)
    sinks = np.asarray(sinks, np.float32)

    ADIM = HQ * HD
    NMT = HQ // 2
    in_maps = []
    for c in range(n_cores):
        qrows = slice(c * ADIM, (c + 1) * ADIM)
        krows = slice(c * HD, (c + 1) * HD)
        wqT = np.ascontiguousarray(wq_w[qrows].T * SCALE).astype(bf16)
        bqv = (wq_b[qrows] * SCALE).reshape(NMT, 128).T  # (128, NMT)
        wkv = np.concatenate([wk_w[krows], wv_w[krows]], 0)  # (128, D)
        wkvT = np.ascontiguousarray(wkv.T).astype(bf16)
        bkv = np.concatenate([wk_b[krows], wv_b[krows]])[:, None]
        woT = np.ascontiguousarray(wo_w[:, qrows].T).astype(bf16)
        es = np.exp(sinks[c * HQ : (c + 1) * HQ])
        esinks = np.repeat(es[None, :], 128, 0)
        csall = np.ascontiguousarray(
            np.concatenate([cosT, sinTs, mask01], axis=1)
        )
        cf = np.ascontiguousarray(
            np.concatenate(
                [bqv.astype(np.float32), bkv.astype(np.float32), esinks], axis=1
            )
        )
        in_maps.append(
            {"xT": xT, "wqT": wqT, "wkvT": wkvT, "woT": woT,
             "csall": csall, "cf": cf}
        )
    return in_maps


_CACHED = {}


def _make_spmd_runner(nc, in_maps, n_cores):
    """Compile the SPMD program via PJRT (axon) and return
    (run_fn, in_arrays) where run_fn(*arrays) executes on the 8 cores and
    returns per-core output dicts. Outputs are NOT donated (our kernel
    writes every element of out), so the device-resident input arrays can
    be reused across calls for warm-run timing."""
    import jax
    from jax.experimental.shard_map import shard_map
    from jax.sharding import Mesh, NamedSharding, PartitionSpec

    from concourse import bass2jax, mybir as mb

    bass2jax.install_neuronx_cc_hook()
    try:
        import libneuronxla

        if not getattr(libneuronxla, "_err_surfacing", False):
            _inner = libneuronxla.neuronx_cc

            def _wrapped(*a, **kw):
                try:
                    return _inner(*a, **kw)
                except Exception:
                    import traceback

                    traceback.print_exc()
                    raise

            libneuronxla.neuronx_cc = _wrapped
            libneuronxla._err_surfacing = True
    except ImportError:
        pass
    assert nc.dbg_addr is None
    partition_name = nc.partition_id_tensor.name if nc.partition_id_tensor else None

    in_names = []
    out_names = []
    out_avals = []
    zero_outs = []
    for alloc in nc.m.functions[0].allocations:
        if not isinstance(alloc, mb.MemoryLocationSet):
            continue
        name = alloc.memorylocations[0].name
        if alloc.kind == "ExternalInput":
            if name != partition_name:
                in_names.append(name)
        elif alloc.kind == "ExternalOutput":
            out_names.append(name)
            shape = tuple(alloc.tensor_shape)
            dtype = mb.dt.np(alloc.dtype)
            out_avals.append(jax.core.ShapedArray(shape, dtype))
            zero_outs.append(np.zeros(shape, dtype))
    n_params = len(in_names)
    all_names = in_names + out_names
    if partition_name is not None:
        all_names = all_names + [partition_name]

    def _body(*args):
        operands = list(args)
        if partition_name is not None:
            operands.append(bass2jax.partition_id_tensor())
        outs = bass2jax._bass_exec_p.bind(
            *operands,
            out_avals=tuple(out_avals),
            in_names=tuple(all_names),
            out_names=tuple(out_names),
            lowering_input_output_aliases=(),
            sim_require_finite=True,
            sim_require_nnan=True,
            nc=nc,
        )
        return tuple(outs)

    devices = jax.devices()[:n_cores]
    mesh = Mesh(np.asarray(devices), ("core",))
    sharded = jax.jit(
        shard_map(
            _body,
            mesh=mesh,
            in_specs=(PartitionSpec("core"),) * (n_params + len(out_names)),
            out_specs=(PartitionSpec("core"),) * len(out_names),
            check_rep=False,
        ),
        keep_unused=True,
    )
    sh = NamedSharding(mesh, PartitionSpec("core"))
    arrs = []
    for i, name in enumerate(in_names):
        cat = np.concatenate([m[name] for m in in_maps], axis=0)
        arrs.append(jax.device_put(cat, sh))
    for z in zero_outs:
        cat = np.zeros((n_cores * z.shape[0], *z.shape[1:]), z.dtype)
        arrs.append(jax.device_put(cat, sh))

    def run():
        import jax as _jax

        return _jax.block_until_ready(sharded(*arrs))

    run.async_call = lambda: sharded(*arrs)

    def unpack(out_arrs):
        return [
            {
                name: np.asarray(out_arrs[i]).reshape(n_cores, *out_avals[i].shape)[c]
                for i, name in enumerate(out_names)
            }
            for c in range(n_cores)
        ]

    return run, unpack


def _tiny_nc():
    """Minimal 8-core program to measure the dispatch/RTT floor."""
    nc = bacc.Bacc(None, target_bir_lowering=False, debug=False)
    a = nc.declare_dram_parameter("a", [128, 128], F32, isOutput=False)
    b = nc.declare_dram_parameter("b", [128, 128], F32, isOutput=True)
    with tile.TileContext(nc) as tc, ExitStack() as ctx:
        pool = ctx.enter_context(tc.tile_pool(name="p", bufs=1))
        t = pool.tile([128, 128], F32)
        nc.sync.dma_start(out=t, in_=a[:, :])
        nc.sync.dma_start(out=b[:, :], in_=t)
    nc.finalize()
    return nc


def measure_overhead_ns(n_warm=10):
    import time

    nc = _tiny_nc()
    in_maps = [{"a": np.zeros((128, 128), np.float32)} for _ in range(N_CORES)]
    run, _ = _make_spmd_runner(nc, in_maps, N_CORES)
    run()
    best = float("inf")
    for _ in range(n_warm):
        t0 = time.perf_counter()
        run()
        best = min(best, time.perf_counter() - t0)
    return best * 1e9


def kernel(x, rope_cache, wq_w, wq_b, wk_w, wk_b, wv_w, wv_b, wo_w, wo_b,
           sinks, sliding_window, _time_runs=0):
    import time

    in_maps = make_core_inputs(
        x, rope_cache, wq_w, wq_b, wk_w, wk_b, wv_w, wv_b, wo_w, sinks
    )
    if "nc" not in _CACHED:
        _CACHED["nc"] = build_nc()
    nc = _CACHED["nc"]
    run, unpack = _make_spmd_runner(nc, in_maps, N_CORES)
    _CACHED["run"] = run
    out_arrs = run()  # compile + first run
    if _time_runs:
        best = float("inf")
        for _ in range(_time_runs):
            t0 = time.perf_counter()
            out_arrs = run()
            best = min(best, time.perf_counter() - t0)
        kernel.last_wall_ns = best * 1e9
    else:
        kernel.last_wall_ns = None
    res = unpack(out_arrs)
    out = None
    for r in res:
        o = np.asarray(r["out"], dtype=np.float32)
        out = o if out is None else out + o
    out = out + np.asarray(wo_b, np.float32)[None, :]
    return out.reshape(1, S_FULL, D_FULL).astype(np.float32)


kernel.last_wall_ns = None


# revision 21
# speedup vs baseline: 7.3207x; 3.0999x over previous
"""Trainium2 Bass kernel: sparse (sliding-window) attention, tensor-parallel
over heads across 8 NeuronCores.

Reference computation (per problem):
  B=1, S=2048, DIM=2880, 64 q heads / 8 kv heads, head_dim 64,
  RoPE (rotate_half, duplicated angles), causal + sliding_window=128 mask,
  logsumexp-sigmoid attention sinks, output projection.

Sharding: core c gets q heads 8c..8c+7 (one kv-head group), wq/wk/wv sliced
column-wise, wo row-wise; each core computes a partial (S, DIM) output which
the host sums (+ wo bias).

Key algebraic simplification: with no max-subtraction in the softmax
(scores are O(+-20) here, safe in fp32),
  out = (softmax(s) @ V) * sigmoid(lse - sink) = (exp(s) @ V) / (l + exp(sink))
where l = sum(exp(s)).  The l column is produced by the PV matmul itself via a
ones-column appended to V.

v2: all operands bf16 (host-side conversion) — halves DMA, removes the
f32->f32r staging copies (DMA lands directly in matmul-ready tiles), runs
small-N matmuls at 1 cyc/row, enables FWL weight loads.  Scores for the
even/odd head halves run as concurrent 64-row tile_position matmuls.
PSUM accumulation stays fp32 throughout.
"""

import math
from contextlib import ExitStack

import numpy as np

import concourse.bass as bass
import concourse.mybir as mybir
import concourse.tile as tile
from concourse import bacc
from concourse.masks import make_identity

F32 = mybir.dt.float32
BF16 = mybir.dt.bfloat16
AF = mybir.ActivationFunctionType

NEG = -1.0e9

# problem constants
S_FULL = 2048
D_FULL = 2880
N_HEADS = 64
N_KV = 8
HD = 64
WINDOW = 128
BLK = 128  # query/key block (must equal WINDOW)
N_CORES = 8
ROPE_FACTOR = 32.0
SCALE = (0.1 * math.log(ROPE_FACTOR) + 1.0) / math.sqrt(HD)


def build_nc(S=S_FULL, D=D_FULL, HQ=N_HEADS // N_CORES, seq_chunk=512):
    """Build the SPMD per-core Bass program.

    Per-core DRAM parameters (host-prepped layouts):
      xT     (D, S)        bf16  input, D on partitions (22.5 128-tiles)
      wqT    (D, HQ*HD)    bf16  scale folded in
      bq     (128, HQ//2)  f32   per-qdim bias columns per 128-row M tile
      wkvT   (D, 128)      bf16  [wk | wv] stacked
      bkv    (128, 1)      f32
      woT    (HQ*HD, D)    bf16
      cosT   (128, S)      bf16  rope cos, rows duplicated x2
      sinTs  (128, S)      bf16  rope sin, signed (-sin for dim<32), dup x2
      mask01 (128, 512)    bf16  0/1 multiplicative mask, scores-transposed
      esinks (128, HQ)     f32   exp(sink_h) replicated down partitions
      out    (S, D)        bf16  output partial
    """
    assert S % BLK == 0 and HQ % 2 == 0
    NQB = S // BLK
    KT = (D + 127) // 128  # D-dim 128-tiles (last may be short)
    NMT = HQ // 2  # 128-row M tiles for Q (2 heads each)
    ADIM = HQ * HD
    seq_chunk = min(seq_chunk, S)
    NSC = (S + seq_chunk - 1) // seq_chunk
    assert S % seq_chunk == 0
    # output D chunks of <=512
    dchunks = []
    off = 0
    while off < D:
        w = min(512, D - off)
        dchunks.append((off, w))
        off += w

    nc = bacc.Bacc(None, target_bir_lowering=False, debug=False)

    # all bf16 inputs live in ONE flat param (minimizes per-call dispatch
    # args in the harness); row-major regions are re-viewed 2D below with
    # byte-identical access patterns
    CS_W = 2 * S + 512
    SEGS = [("xT", D, S), ("wqT", D, ADIM), ("wkvT", D, 128),
            ("woT", ADIM, D), ("csall", 128, CS_W)]
    NTOT = sum(r * c for _, r, c in SEGS)
    flat_d = nc.declare_dram_parameter("flat", [1, NTOT], BF16, isOutput=False)
    views = {}
    _off = 0
    for nm, r, c in SEGS:
        views[nm] = flat_d[0:1, _off : _off + r * c].rearrange(
            "o (r c) -> (o r) c", c=c
        )
        _off += r * c
    xT_d = views["xT"]
    wqT_d = views["wqT"]
    wkvT_d = views["wkvT"]
    woT_d = views["woT"]
    csall_d = views["csall"]
    # cf packs [bq (NMT) | bkv (1) | esinks (HQ)] f32 columns
    cf_d = nc.declare_dram_parameter("cf", [128, NMT + 1 + HQ], F32, isOutput=False)
    out_d = nc.declare_dram_parameter("out", [S, D], BF16, isOutput=True)

    nfull = D // 128
    rem = D % 128
    GX = 2  # kt-tiles per grouped x DMA (small groups pace PE startup)
    xgroups = [(g, min(GX, nfull - g)) for g in range(0, nfull, GX)]
    GW = 2
    wgroups = [(g, min(GW, nfull - g)) for g in range(0, nfull, GW)]
    GKV = 11
    kvgroups = [(g, min(GKV, nfull - g)) for g in range(0, nfull, GKV)]

    with tile.TileContext(nc) as tc, ExitStack() as ctx:
        # ---------------- persistent pools ----------------
        const = ctx.enter_context(tc.tile_pool(name="const", bufs=1))
        qkpool = ctx.enter_context(tc.tile_pool(name="qkpool", bufs=1))
        psum_proj = ctx.enter_context(tc.tile_pool(name="psum_proj", bufs=2, space="PSUM"))
        psum_s = ctx.enter_context(tc.tile_pool(name="psum_s", bufs=3, space="PSUM"))
        psum_o = ctx.enter_context(tc.tile_pool(name="psum_o", bufs=2, space="PSUM"))
        psum_t = ctx.enter_context(tc.tile_pool(name="psum_t", bufs=1, space="PSUM"))

        csall = const.tile([128, 2 * S + 512], BF16)
        cosT = csall[:, 0:S]
        sinTs = csall[:, S : 2 * S]
        mask01 = csall[:, 2 * S : 2 * S + 512]
        cf = const.tile([128, NMT + 1 + HQ], F32)
        bq = cf[:, 0:NMT]
        bkv = cf[:, NMT : NMT + 1]
        esinks = cf[:, NMT + 1 : NMT + 1 + HQ]
        identF = const.tile([128, 128], F32)
        ident = const.tile([128, 128], BF16)
        ones2 = const.tile([128, 2], BF16)

        # persistent activations
        qts = []
        for t in range(NMT):
            qt = qkpool.tile([128, S], BF16, name=f"qt{t}", tag=f"qt{t}")
            qts.append(qt)
        kvt = qkpool.tile([128, S], BF16, name="kvt", tag="kvt")
        kpadO = qkpool.tile([128, S], BF16, name="kpadO", tag="kpadO")
        vaug = []
        for kb in range(NQB):
            va = qkpool.tile([128, HD + 2], BF16, name=f"vaug{kb}", tag=f"vaug{kb}")
            vaug.append(va)
        # wo resident for the whole kernel; DMAs queue behind the phase-1
        # loads and complete long before the first out-projection needs them
        wo_tiles = []
        for t2i in range(ADIM // 128):
            w = qkpool.tile([128, D], BF16, name=f"wo{t2i}", tag=f"wo{t2i}")
            nc.sync.dma_start(out=w, in_=woT_d[t2i * 128 : (t2i + 1) * 128, :])
            wo_tiles.append(w)

        # ---------------- phase 1: QKV projection ----------------
        with tc.tile_pool(name="ph1", bufs=1) as ph1, \
             tc.tile_pool(name="xpool", bufs=1) as xpool, \
             tc.tile_pool(name="ropetmp", bufs=1) as ropetmp:
            # big flat weight/x tiles: col = kt*width + inner
            wqbig = ph1.tile([128, nfull * ADIM], BF16, name="wqbig", tag="wqbig")
            wqlast = ph1.tile([rem, ADIM], BF16, name="wqlast", tag="wqlast") if rem else None
            wkvbig = ph1.tile([128, nfull * 128], BF16, name="wkvbig", tag="wkvbig")
            wkvlast = ph1.tile([rem, 128], BF16, name="wkvlast", tag="wkvlast") if rem else None

            def load_x_chunk(nt):
                xbig = xpool.tile(
                    [128, nfull * seq_chunk], BF16, name=f"xbig{nt}", tag="xbig", bufs=2
                )
                xlast = (
                    xpool.tile([rem, seq_chunk], BF16, name=f"xlast{nt}", tag="xlast", bufs=2)
                    if rem else None
                )
                c0 = nt * seq_chunk
                for gi, (g0, g) in enumerate(xgroups):
                    nc.sync.dma_start(
                        out=xbig[:, g0 * seq_chunk : (g0 + g) * seq_chunk].rearrange(
                            "p (g s) -> p g s", g=g
                        ),
                        in_=xT_d[g0 * 128 : (g0 + g) * 128, c0 : c0 + seq_chunk].rearrange(
                            "(g p) s -> p g s", p=128
                        ),
                    )
                    if nt == 0:
                        load_wq_group(gi)
                if rem:
                    nc.sync.dma_start(
                        out=xlast, in_=xT_d[nfull * 128 : D, c0 : c0 + seq_chunk]
                    )
                return xbig, xlast

            _wq_loaded = set()

            def load_wq_group(gi):
                # interleave wq loads with chunk-0 x loads, one wq group per
                # x group (wgroups and xgroups have the same count)
                if gi >= len(wgroups) or gi in _wq_loaded:
                    return
                _wq_loaded.add(gi)
                g0, g = wgroups[gi]
                nc.sync.dma_start(
                    out=wqbig[:, g0 * ADIM : (g0 + g) * ADIM].rearrange(
                        "p (g m) -> p g m", g=g
                    ),
                    in_=wqT_d[g0 * 128 : (g0 + g) * 128, :].rearrange(
                        "(g p) m -> p g m", p=128
                    ),
                )

            xchunk0 = load_x_chunk(0)  # interleaves wq group loads
            for gi in range(len(wgroups)):
                load_wq_group(gi)
            if rem:
                nc.sync.dma_start(out=wqlast, in_=wqT_d[nfull * 128 : D, :])
            nc.sync.dma_start(out=cf, in_=cf_d[:, :])
            for g0, g in kvgroups:
                nc.sync.dma_start(
                    out=wkvbig[:, g0 * 128 : (g0 + g) * 128].rearrange(
                        "p (g m) -> p g m", g=g
                    ),
                    in_=wkvT_d[g0 * 128 : (g0 + g) * 128, :].rearrange(
                        "(g p) m -> p g m", p=128
                    ),
                )
            if rem:
                nc.sync.dma_start(out=wkvlast, in_=wkvT_d[nfull * 128 : D, :])
            nc.sync.dma_start(out=csall, in_=csall_d[:, :])
            make_identity(nc, identF)
            nc.gpsimd.tensor_copy(ident, identF)
            nc.vector.memset(ones2, 1.0)

            for nt in range(NSC):
                c0 = nt * seq_chunk
                xbig, xlast = xchunk0 if nt == 0 else load_x_chunk(nt)
                # kt-outer / mt-inner: each x tile is consumed right after its
                # DMA lands, with 5 concurrent PSUM accumulation groups
                pss = []
                for mt in range(NMT + 1):
                    pool = psum_proj if mt < 2 else psum_s
                    pss.append(
                        pool.tile(
                            [128, seq_chunk], F32, name=f"psp_{nt}_{mt}",
                            tag="proj" if mt < 2 else "s",
                        )
                    )
                for kt in range(KT):
                    if kt < nfull:
                        rhs = xbig[:, kt * seq_chunk : (kt + 1) * seq_chunk]
                    else:
                        rhs = xlast
                    for mt in range(NMT + 1):
                        if kt < nfull:
                            if mt < NMT:
                                lhs = wqbig[:, kt * ADIM + mt * 128 : kt * ADIM + (mt + 1) * 128]
                            else:
                                lhs = wkvbig[:, kt * 128 : (kt + 1) * 128]
                        else:
                            lhs = wqlast[:, mt * 128 : (mt + 1) * 128] if mt < NMT else wkvlast
                        nc.tensor.matmul(
                            pss[mt], lhs, rhs, start=(kt == 0), stop=(kt == KT - 1)
                        )
                for mt in range(NMT + 1):
                    if mt < NMT:
                        dst = qts[mt]
                        bias = bq[:, mt : mt + 1]
                    else:
                        dst = kvt
                        bias = bkv[:, 0:1]
                    nc.scalar.activation(
                        dst[:, c0 : c0 + seq_chunk], pss[mt], AF.Identity, bias=bias
                    )

                # rope on this seq chunk (swap halves via gpsimd partition-offset copies)
                for t in range(NMT + 1):
                    if t < NMT:
                        src = qts[t]
                        npart = 128
                    else:
                        src = kvt
                        npart = 64  # k rows only
                    sw = ropetmp.tile([128, seq_chunk], BF16, name=f"sw_{nt}_{t}", tag="sw")
                    for base in range(0, npart, 64):
                        nc.gpsimd.tensor_copy(
                            sw[base : base + 32, :],
                            src[base + 32 : base + 64, c0 : c0 + seq_chunk],
                        )
                        nc.gpsimd.tensor_copy(
                            sw[base + 32 : base + 64, :],
                            src[base : base + 32, c0 : c0 + seq_chunk],
                        )
                    t2 = ropetmp.tile([128, seq_chunk], BF16, name=f"t2_{nt}_{t}", tag="t2")
                    nc.vector.tensor_mul(
                        t2[:npart], sw[:npart], sinTs[:npart, c0 : c0 + seq_chunk]
                    )
                    nc.vector.tensor_mul(
                        src[:npart, c0 : c0 + seq_chunk],
                        src[:npart, c0 : c0 + seq_chunk],
                        cosT[:npart, c0 : c0 + seq_chunk],
                    )
                    nc.vector.tensor_add(
                        src[:npart, c0 : c0 + seq_chunk],
                        src[:npart, c0 : c0 + seq_chunk],
                        t2[:npart],
                    )

                # kpadO: K replicated to partitions 64..127, so the odd heads'
                # scores matmul runs as a concurrent (64,0)-row-tile matmul
                cs = slice(c0, c0 + seq_chunk)
                nc.gpsimd.tensor_copy(kpadO[64:128, cs], kvt[0:64, cs])
                # V natural (+ ones cols) per key block in this chunk
                for kb in range(c0 // BLK, (c0 + seq_chunk) // BLK):
                    ptv = psum_t.tile([128, 128], BF16, name=f"vtr{kb}", tag="tr")
                    nc.tensor.transpose(
                        ptv[:, 0:64],
                        kvt[64:128, kb * BLK : (kb + 1) * BLK],
                        ident[64:128, 64:128],
                    )
                    nc.scalar.copy(vaug[kb][:, 0:HD], ptv[:, 0:64])
                    nc.gpsimd.tensor_copy(vaug[kb][:, HD : HD + 2], ones2)

        # ---------------- phase 1.6 + 2 + 3 pools ----------------
        with tc.tile_pool(name="att", bufs=1) as att, \
             tc.tile_pool(name="ppool", bufs=3) as ppool, \
             tc.tile_pool(name="onat_pool", bufs=3) as onat_pool, \
             tc.tile_pool(name="small", bufs=16) as small, \
             tc.tile_pool(name="stage", bufs=2) as stage:

            # ---------------- phase 2+3: attention + out projection ----------------
            p_prev = [None] * (HQ // 2)
            for qb in range(NQB):
                ncols = 256 if qb < NQB - 1 else 128
                onats = []
                for hp in range(HQ // 2):
                    h0 = 2 * hp
                    qtile = qts[hp]
                    # paired scores^T: even head in cols 0:256, odd in 256:512
                    # of pt. The two matmuls run concurrently as 64-row
                    # tile_position groups — their outputs MUST live in
                    # different PSUM banks (same-bank concurrent writes hang
                    # the device).
                    ps_sA = psum_s.tile([128, 256], F32, name=f"sA_{qb}_{hp}", tag="s")
                    ps_sB = psum_s.tile([128, 256], F32, name=f"sB_{qb}_{hp}", tag="s")
                    nc.tensor.matmul(
                        ps_sA[:, 0:ncols],
                        kvt[0:64, qb * BLK : (qb + 1) * BLK],
                        qtile[0:64, qb * BLK : qb * BLK + ncols],
                        start=True,
                        stop=True,
                    )
                    nc.tensor.matmul(
                        ps_sB[:, 0:ncols],
                        kpadO[64:128, qb * BLK : (qb + 1) * BLK],
                        qtile[64:128, qb * BLK : qb * BLK + ncols],
                        start=True,
                        stop=True,
                    )
                    pt = ppool.tile([128, 512], BF16, name=f"p_{qb}_{hp}", tag=f"pp{hp}")
                    for po_, pss_ in ((0, ps_sA), (256, ps_sB)):
                        nc.scalar.activation(
                            pt[:, po_ : po_ + ncols], pss_[:, 0:ncols], AF.Exp
                        )
                        nc.gpsimd.tensor_mul(
                            pt[:, po_ : po_ + ncols],
                            pt[:, po_ : po_ + ncols],
                            mask01[:, po_ : po_ + ncols],
                        )

                    onat = onat_pool.tile(
                        [128, 128], BF16, name=f"on_{qb}_{hp}", tag="onat", bufs=HQ
                    )
                    onats.append(onat)
                    ps_po = psum_o.tile(
                        [128, 2 * (HD + 2)], F32, name=f"o_{qb}_{hp}", tag="o"
                    )
                    for hh in range(2):
                        po = 256 * hh
                        oo = (HD + 2) * hh
                        dst = ps_po[:, oo : oo + HD + 2]
                        if qb > 0:
                            nc.tensor.matmul(
                                dst,
                                p_prev[hp][:, po + 128 : po + 256],
                                vaug[qb - 1],
                                start=True,
                                stop=False,
                            )
                            nc.tensor.matmul(
                                dst,
                                pt[:, po : po + 128],
                                vaug[qb],
                                start=False,
                                stop=True,
                            )
                        else:
                            nc.tensor.matmul(
                                dst,
                                pt[:, po : po + 128],
                                vaug[0],
                                start=True,
                                stop=True,
                            )
                    # denom = l + exp(sink); r = 1/denom; o = o_un * r
                    dn = small.tile([128, 2], F32, name=f"dn_{qb}_{hp}", tag="dn")
                    rr = small.tile([128, 2], F32, name=f"rr_{qb}_{hp}", tag="rr")
                    for hh in range(2):
                        oo = (HD + 2) * hh
                        nc.vector.tensor_add(
                            dn[:, hh : hh + 1],
                            ps_po[:, oo + HD : oo + HD + 1],
                            esinks[:, h0 + hh : h0 + hh + 1],
                        )
                    nc.vector.reciprocal(rr, dn)
                    for hh in range(2):
                        oo = (HD + 2) * hh
                        nc.vector.tensor_scalar_mul(
                            onat[:, 64 * hh : 64 * hh + 64],
                            ps_po[:, oo : oo + HD],
                            rr[:, hh : hh + 1],
                        )
                    p_prev[hp] = pt

                # transpose head pairs into OT layout, then out projection
                ot_cols = []
                for t2i in range(HQ // 2):
                    ptr = psum_t.tile([128, 128], BF16, name=f"otr_{qb}_{t2i}", tag="tr")
                    nc.tensor.transpose(ptr, onats[t2i], ident)
                    otc = onat_pool.tile(
                        [128, 128], BF16, name=f"otc_{qb}_{t2i}", tag="otc", bufs=HQ
                    )
                    nc.scalar.copy(otc, ptr)
                    ot_cols.append(otc)

                ost = stage.tile([128, D], BF16, name=f"ost_{qb}", tag="ost")
                for dc, (doff, dw) in enumerate(dchunks):
                    ps = psum_proj.tile([128, dw], F32, name=f"po_{qb}_{dc}", tag="proj")
                    for t2i in range(HQ // 2):
                        nc.tensor.matmul(
                            ps[:, :dw],
                            ot_cols[t2i],
                            wo_tiles[t2i][:, doff : doff + dw],
                            start=(t2i == 0),
                            stop=(t2i == HQ // 2 - 1),
                        )
                    nc.vector.tensor_copy(ost[:, doff : doff + dw], ps[:, :dw])
                nc.sync.dma_start(out=out_d[qb * BLK : (qb + 1) * BLK, :], in_=ost)

    nc.finalize()
    return nc


def make_core_inputs(x, rope_cache, wq_w, wq_b, wk_w, wk_b, wv_w, wv_b, wo_w,
                     sinks, S=S_FULL, D=D_FULL, HQ=N_HEADS // N_CORES,
                     n_cores=N_CORES):
    """Host-side prep: build the per-core input maps (bf16 activations)."""
    import ml_dtypes

    bf16 = ml_dtypes.bfloat16

    x2 = np.asarray(x, np.float32).reshape(S, D)
    xT = np.ascontiguousarray(x2.T).astype(bf16)

    rc = np.asarray(rope_cache, np.float32)
    cos = rc[:S, :HD].T  # (64, S)
    sin = rc[:S, HD:].T
    cosT = np.ascontiguousarray(np.concatenate([cos, cos], 0)).astype(bf16)
    sgn = np.concatenate([-np.ones((32, 1), np.float32), np.ones((32, 1), np.float32)])
    sinTs = np.ascontiguousarray(np.concatenate([sin * sgn, sin * sgn], 0)).astype(bf16)

    m256 = np.zeros((128, 256), np.float32)
    kk = np.arange(128)[:, None]
    cc = np.arange(128)[None, :]
    m256[:, :128] = np.where(kk <= cc, 1.0, 0.0)
    m256[:, 128:] = np.where(kk > cc, 1.0, 0.0)
    mask01 = np.concatenate([m256, m256], axis=1).astype(bf16)  # (128,512), head pair

    wq_w = np.asarray(wq_w, np.float32)
    wq_b = np.asarray(wq_b, np.float32)
    wk_w = np.asarray(wk_w, np.float32)
    wk_b = np.asarray(wk_b, np.float32)
    wv_w = np.asarray(wv_w, np.float32)
    wv_b = np.asarray(wv_b, np.float32)
    wo_w = np.asarray(wo_w, np.float32)
    sinks = np.asarray(sinks, np.float32)

    ADIM = HQ * HD
    NMT = HQ // 2
    in_maps = []
    for c in range(n_cores):
        qrows = slice(c * ADIM, (c + 1) * ADIM)
        krows = slice(c * HD, (c + 1) * HD)
        wqT = np.ascontiguousarray(wq_w[qrows].T * SCALE).astype(bf16)
        bqv = (wq_b[qrows] * SCALE).reshape(NMT, 128).T  # (128, NMT)
        wkv = np.concatenate([wk_w[krows], wv_w[krows]], 0)  # (128, D)
        wkvT = np.ascontiguousarray(wkv.T).astype(bf16)
        bkv = np.concatenate([wk_b[krows], wv_b[krows]])[:, None]
        woT = np.ascontiguousarray(wo_w[:, qrows].T).astype(bf16)
        es = np.exp(sinks[c * HQ : (c + 1) * HQ])
        esinks = np.repeat(es[None, :], 128, 0)
        csall = np.ascontiguousarray(
            np.concatenate([cosT, sinTs, mask01], axis=1)
        )
        cf = np.ascontiguousarray(
            np.concatenate(
                [bqv.astype(np.float32), bkv.astype(np.float32), esinks], axis=1
            )
        )
        flat = np.concatenate(
            [xT.ravel(), wqT.ravel(), wkvT.ravel(), woT.ravel(), csall.ravel()]
        )[None, :]
        in_maps.append({"flat": np.ascontiguousarray(flat), "cf": cf})
    return in_maps


_CACHED = {}


def _make_spmd_runner(nc, in_maps, n_cores):
    """Compile the SPMD program via PJRT (axon) and return
    (run_fn, in_arrays) where run_fn(*arrays) executes on the 8 cores and
    returns per-core output dicts. Outputs are NOT donated (our kernel
    writes every element of out), so the device-resident input arrays can
    be reused across calls for warm-run timing."""
    import jax
    from jax.experimental.shard_map import shard_map
    from jax.sharding import Mesh, NamedSharding, PartitionSpec

    from concourse import bass2jax, mybir as mb

    bass2jax.install_neuronx_cc_hook()
    try:
        import libneuronxla

        if not getattr(libneuronxla, "_err_surfacing", False):
            _inner = libneuronxla.neuronx_cc

            def _wrapped(*a, **kw):
                try:
                    return _inner(*a, **kw)
                except Exception:
                    import traceback

                    traceback.print_exc()
                    raise

            libneuronxla.neuronx_cc = _wrapped
            libneuronxla._err_surfacing = True
    except ImportError:
        pass
    assert nc.dbg_addr is None
    partition_name = nc.partition_id_tensor.name if nc.partition_id_tensor else None

    in_names = []
    out_names = []
    out_avals = []
    zero_outs = []
    for alloc in nc.m.functions[0].allocations:
        if not isinstance(alloc, mb.MemoryLocationSet):
            continue
        name = alloc.memorylocations[0].name
        if alloc.kind == "ExternalInput":
            if name != partition_name:
                in_names.append(name)
        elif alloc.kind == "ExternalOutput":
            out_names.append(name)
            shape = tuple(alloc.tensor_shape)
            dtype = mb.dt.np(alloc.dtype)
            out_avals.append(jax.core.ShapedArray(shape, dtype))
            zero_outs.append(np.zeros(shape, dtype))
    n_params = len(in_names)
    all_names = in_names + out_names
    if partition_name is not None:
        all_names = all_names + [partition_name]

    def _body(*args):
        operands = list(args)
        if partition_name is not None:
            operands.append(bass2jax.partition_id_tensor())
        outs = bass2jax._bass_exec_p.bind(
            *operands,
            out_avals=tuple(out_avals),
            in_names=tuple(all_names),
            out_names=tuple(out_names),
            lowering_input_output_aliases=(),
            sim_require_finite=True,
            sim_require_nnan=True,
            nc=nc,
        )
        return tuple(outs)

    devices = jax.devices()[:n_cores]
    mesh = Mesh(np.asarray(devices), ("core",))
    sharded = jax.jit(
        shard_map(
            _body,
            mesh=mesh,
            in_specs=(PartitionSpec("core"),) * (n_params + len(out_names)),
            out_specs=(PartitionSpec("core"),) * len(out_names),
            check_rep=False,
        ),
        keep_unused=True,
    )
    sh = NamedSharding(mesh, PartitionSpec("core"))
    arrs = []
    for i, name in enumerate(in_names):
        cat = np.concatenate([m[name] for m in in_maps], axis=0)
        arrs.append(jax.device_put(cat, sh))
    for z in zero_outs:
        cat = np.zeros((n_cores * z.shape[0], *z.shape[1:]), z.dtype)
        arrs.append(jax.device_put(cat, sh))

    def run():
        import jax as _jax

        return _jax.block_until_ready(sharded(*arrs))

    run.async_call = lambda: sharded(*arrs)

    def unpack(out_arrs):
        return [
            {
                name: np.asarray(out_arrs[i]).reshape(n_cores, *out_avals[i].shape)[c]
                for i, name in enumerate(out_names)
            }
            for c in range(n_cores)
        ]

    return run, unpack


def _tiny_nc():
    """Minimal 8-core program to measure the dispatch/RTT floor."""
    nc = bacc.Bacc(None, target_bir_lowering=False, debug=False)
    a = nc.declare_dram_parameter("a", [128, 128], F32, isOutput=False)
    b = nc.declare_dram_parameter("b", [128, 128], F32, isOutput=True)
    with tile.TileContext(nc) as tc, ExitStack() as ctx:
        pool = ctx.enter_context(tc.tile_pool(name="p", bufs=1))
        t = pool.tile([128, 128], F32)
        nc.sync.dma_start(out=t, in_=a[:, :])
        nc.sync.dma_start(out=b[:, :], in_=t)
    nc.finalize()
    return nc


def measure_overhead_ns(n_warm=10):
    import time

    nc = _tiny_nc()
    in_maps = [{"a": np.zeros((128, 128), np.float32)} for _ in range(N_CORES)]
    run, _ = _make_spmd_runner(nc, in_maps, N_CORES)
    run()
    best = float("inf")
    for _ in range(n_warm):
        t0 = time.perf_counter()
        run()
        best = min(best, time.perf_counter() - t0)
    return best * 1e9


def kernel(x, rope_cache, wq_w, wq_b, wk_w, wk_b, wv_w, wv_b, wo_w, wo_b,
           sinks, sliding_window, _time_runs=0):
    import time

    in_maps = make_core_inputs(
        x, rope_cache, wq_w, wq_b, wk_w, wk_b, wv_w, wv_b, wo_w, sinks
    )
    if "nc" not in _CACHED:
        _CACHED["nc"] = build_nc()
    nc = _CACHED["nc"]
    run, unpack = _make_spmd_runner(nc, in_maps, N_CORES)
    _CACHED["run"] = run
    out_arrs = run()  # compile + first run
    if _time_runs:
        best = float("inf")
        for _ in range(_time_runs):
            t0 = time.perf_counter()
            out_arrs = run()
            best = min(best, time.perf_counter() - t0)
        kernel.last_wall_ns = best * 1e9
    else:
        kernel.last_wall_ns = None
    res = unpack(out_arrs)
    out = None
    for r in res:
        o = np.asarray(r["out"], dtype=np.float32)
        out = o if out is None else out + o
    out = out + np.asarray(wo_b, np.float32)[None, :]
    return out.reshape(1, S_FULL, D_FULL).astype(np.float32)


kernel.last_wall_ns = None
